# revision 25
# baseline (speedup 1.0000x reference)
"""Trainium2 Bass kernel for DGL HGNNConv-style hypergraph message passing.

Computation (see problem reference):
    Xp = X @ Wlin                                   # [N, 128] @ [128, 128]
    Xe = segment_sum(Xp[g1_src], g1_dst, 25000)     # node -> hyperedge
    Xe = Xe * degE * W
    Xv = segment_sum(Xe[g1_dst], g1_src, 100000)    # hyperedge -> node
    Xv = Xv * degV

Distribution strategy (8 NeuronCores, node-range sharding):
  - Core m owns node rows [m*12500, (m+1)*12500) and all nnz entries whose
    src falls in that range (both stages use the same entry sharding).
  - Projection: each core computes Xp (bf16) for its own node shard only.
    The host ships X pre-transposed (feature-major) in bf16, so projection
    is a plain matmul per tile (no on-chip transpose or cast).
  - Stage 1: per-core entries sorted by dst; rows of the local Xp gathered
    per entry (dma_gather), segment-summed into a full-range partial Xe
    via data-dependent one-hot matmuls (PSUM accumulation), scaled by
    degE*W, then AllReduced across cores (4 chunks, overlapped).
  - Stage 2: per-core entries sorted by src; rows of the reduced Xe
    gathered per entry, segment-summed into the core's node tile,
    scaled by degV, written to the core's output shard.

Performance notes:
  - SWDGE descriptor generation on the Pool engine costs ~1us fixed per
    dma_gather call; calls are packed to exactly CALL_CHUNKS chunks
    (spanning output-tile boundaries), the per-call maximum imposed by the
    ucode's 128-entry in-flight descriptor-ring limit (each ring entry
    covers 16 rows, both directions counted: 7*128/16*2 = 114 <= 128).
    Larger calls deadlock the device.
  - Gather tables (Xp, reduced Xe) are stored bf16: halves gather HBM
    traffic and feeds the one-hot matmuls bf16 inputs directly.
  - One-hot build (is_equal) runs on the vector engine, one instruction
    per gather call; PSUM evacuation + deg scaling runs fused on the
    (otherwise idle) scalar engine via activation(Copy, scale=...).
  - PSUM accumulator chains span gather calls: a tile's matmul chain
    continues into the next call's chunks (start/stop flags bracket the
    whole tile).
  - The AllReduce runs in 3 chunks ([64, 88, 44] seg tiles), each emitted
    2 gather calls into the NEXT chunk's quarter so its link time hides
    behind that quarter's remaining gather stream (shape and emission
    point must be tuned together). Each chunk lands in its own Shared
    tensor (the framework allows only one writer per Shared tensor) and
    is copied into the contiguous xe_full gather table off the critical
    path.

Segment-sum-as-matmul: for each chunk of 128 gathered rows G [128e x 128f]
and one-hot S [128e x 128s] (S[k, m] = 1 iff entry k belongs to local
segment m, built on-chip with is_equal against an iota tile), the matmul
S^T @ G accumulates the chunk into the 128-segment PSUM tile.
"""

import ml_dtypes
import numpy as np

import concourse.bass as bass
import concourse.bacc as bacc
import concourse.tile as tile
import concourse.mybir as mybir
from concourse.bass_utils import run_bass_kernel_spmd

P = 128
N_CORES = 8
N_QUEUES = 4

N_NODES = 100000
N_HEDGES = 25000
IN_CH = 128
OUT_CH = 128
N_AR_CHUNKS = 3  # AllReduce split for overlap with stage-1 compute
USE_COLLECTIVE = True
CALL_CHUNKS = 7  # chunks (x128 rows) per dma_gather call (in-flight limit)

BF16 = ml_dtypes.bfloat16


def _cdiv(a, b):
    return (a + b - 1) // b


def _wrap_idx16(idx_flat: np.ndarray) -> np.ndarray:
    """Pack a flat index array into the [128, n/16] int16 SBUF layout used
    by dma_gather: flat index i -> partition i%16, column i//16, replicated
    across the eight 16-partition stripes."""
    n = idx_flat.shape[0]
    assert n % 16 == 0
    blk = idx_flat.astype(np.int16).reshape(n // 16, 16).T  # [16, cols]
    return np.tile(blk, (8, 1))  # [128, cols]


def _prep_stage(tile_key, gather_idx, local_id, n_tiles, n_cores):
    """Build per-core padded gather-index / segment-id arrays with a chunk
    schedule that is uniform across cores (SPMD requires one program).

    tile_key: per-core arrays with the tile id per entry (nondecreasing).
    Returns (chunks [n_tiles], idx_wrapped list, ids list).
    """
    counts = np.zeros((n_cores, n_tiles), dtype=np.int64)
    slices = []
    for c in range(n_cores):
        bounds = np.searchsorted(tile_key[c], np.arange(n_tiles + 1),
                                 side="left")
        counts[c] = bounds[1:] - bounds[:-1]
        slices.append(bounds)
    chunks = np.maximum(1, _cdiv(counts.max(axis=0), P)).astype(np.int64)
    total_chunks = int(chunks.sum())
    total = total_chunks * P
    co = np.concatenate([[0], np.cumsum(chunks)])

    idx_w, ids_w = [], []
    for c in range(n_cores):
        idx_flat = np.zeros(total, dtype=np.int16)
        ids_flat = np.full(total, -1.0, dtype=np.float32)
        bounds = slices[c]
        gi, li = gather_idx[c], local_id[c]
        for t in range(n_tiles):
            lo, hi = bounds[t], bounds[t + 1]
            cnt = hi - lo
            base = int(co[t]) * P
            idx_flat[base:base + cnt] = gi[lo:hi]
            ids_flat[base:base + cnt] = li[lo:hi]
        idx_w.append(_wrap_idx16(idx_flat))
        ids_w.append(np.ascontiguousarray(
            ids_flat.reshape(total_chunks, P).T).astype(BF16))
    return chunks, idx_w, ids_w


def _make_calls(chunk_lo, chunk_hi, max_chunks):
    """Pack the chunk range [chunk_lo, chunk_hi) into gather calls of at
    most max_chunks chunks (full-size except the last)."""
    calls = []
    c = chunk_lo
    while c < chunk_hi:
        n = min(max_chunks, chunk_hi - c)
        calls.append((c, n))
        c += n
    return calls


def _build_program(ns_pad, seg_pad, chunks1, chunks2, n_cores):
    """Emit the SPMD Bass program (identical for all cores)."""
    n_tiles_proj = ns_pad // P
    n_seg_tiles = seg_pad // P
    n_node_tiles = ns_pad // P
    tc1 = int(chunks1.sum())
    tc2 = int(chunks2.sum())
    co1 = np.concatenate([[0], np.cumsum(chunks1)]).astype(int)
    co2 = np.concatenate([[0], np.cumsum(chunks2)]).astype(int)

    nc = bacc.Bacc("TRN2", target_bir_lowering=False, debug=False,
                   num_devices=n_cores, num_swdge_queues=N_QUEUES)

    xt_shard = nc.dram_tensor("xt_shard", [IN_CH, ns_pad], mybir.dt.bfloat16,
                              kind="ExternalInput")
    wlin = nc.dram_tensor("wlin", [IN_CH, OUT_CH], mybir.dt.bfloat16,
                          kind="ExternalInput")
    dege_r = nc.dram_tensor("dege_r", [P, n_seg_tiles], mybir.dt.float32,
                            kind="ExternalInput")
    degv_r = nc.dram_tensor("degv_r", [P, n_node_tiles], mybir.dt.float32,
                            kind="ExternalInput")
    colidx_in = nc.dram_tensor("colidx", [P, P], mybir.dt.bfloat16,
                               kind="ExternalInput")
    idx1_in = nc.dram_tensor("idx1", [P, tc1 * 8], mybir.dt.int16,
                             kind="ExternalInput")
    ids1_in = nc.dram_tensor("ids1", [P, tc1], mybir.dt.bfloat16,
                             kind="ExternalInput")
    idx2_in = nc.dram_tensor("idx2", [P, tc2 * 8], mybir.dt.int16,
                             kind="ExternalInput")
    ids2_in = nc.dram_tensor("ids2", [P, tc2], mybir.dt.bfloat16,
                             kind="ExternalInput")
    out_shard = nc.dram_tensor("out_shard", [ns_pad, OUT_CH],
                               mybir.dt.float32, kind="ExternalOutput")

    # AllReduce chunk row ranges (in seg tiles): front-loaded groups with a
    # small final group so the pipeline-drain tail before the last AllReduce
    # is short
    n_ar = min(N_AR_CHUNKS, n_seg_tiles)
    if n_ar == 3 and n_seg_tiles == 196:
        q_tiles = [64, 88, 44]
    elif n_ar == 4 and n_seg_tiles == 196:
        q_tiles = [60, 60, 60, 16]
    else:
        q_tiles = [n_seg_tiles // n_ar] * n_ar
        for i in range(n_seg_tiles % n_ar):
            q_tiles[i] += 1
    q_tile_lo = np.concatenate([[0], np.cumsum(q_tiles)]).astype(int)

    # gather calls: exactly CALL_CHUNKS chunks each (spanning tile
    # boundaries), aligned to AR quarters in stage 1
    calls1 = [
        _make_calls(int(co1[q_tile_lo[q]]), int(co1[q_tile_lo[q + 1]]),
                    CALL_CHUNKS)
        for q in range(n_ar)
    ]
    calls2 = _make_calls(0, tc2, CALL_CHUNKS)
    call_sizes = sorted(
        {n for qc in calls1 for (_, n) in qc} | {n for (_, n) in calls2})

    qctr = [0]  # SWDGE queue rotation

    with tile.TileContext(nc) as tc:
        with (
            tc.tile_pool(name="const", bufs=1) as cpool,
            tc.tile_pool(name="work", bufs=16) as work,
            tc.tile_pool(name="evp", bufs=12) as evp,
            tc.tile_pool(name="small", bufs=6) as small,
            tc.tile_pool(name="psum", bufs=2, space="PSUM") as psum,
            tc.tile_pool(name="psacc", bufs=6, space="PSUM") as psacc,
            tc.tile_pool(name="dram", bufs=1, space="DRAM") as dram,
        ):
            # ---- preloads ----
            idx1_sb = cpool.tile([P, tc1 * 8], mybir.dt.int16)
            nc.sync.dma_start(idx1_sb[:], idx1_in[:])
            ids1_sb = cpool.tile([P, tc1], mybir.dt.bfloat16)
            nc.sync.dma_start(ids1_sb[:], ids1_in[:])
            idx2_sb = cpool.tile([P, tc2 * 8], mybir.dt.int16)
            nc.sync.dma_start(idx2_sb[:], idx2_in[:])
            ids2_sb = cpool.tile([P, tc2], mybir.dt.bfloat16)
            nc.sync.dma_start(ids2_sb[:], ids2_in[:])
            colidx_sb = cpool.tile([P, P], mybir.dt.bfloat16)
            nc.sync.dma_start(colidx_sb[:], colidx_in[:])
            wlin_sb = cpool.tile([P, OUT_CH], mybir.dt.bfloat16)
            nc.sync.dma_start(wlin_sb[:], wlin[:])
            degv_sb = cpool.tile([P, n_node_tiles], mybir.dt.float32)
            nc.sync.dma_start(degv_sb[:], degv_r[:])
            scale_e = cpool.tile([P, n_seg_tiles], mybir.dt.float32)
            nc.sync.dma_start(scale_e[:], dege_r[:])
            colidx3 = colidx_sb[:].rearrange("p (o e) -> p o e", o=1)
            # pre-staged num_idxs registers (a MOVE occupies a scarce gpsimd
            # exec-queue slot, so these are hoisted out of the loops)
            nidx_regs = {n: nc.gpsimd.to_reg(n * P) for n in call_sizes}

            xp_local = dram.tile([ns_pad, OUT_CH], mybir.dt.bfloat16)
            xe_part = [
                dram.tile([q_tiles[q] * P, OUT_CH], mybir.dt.bfloat16,
                          name=f"xe_part{q}")
                for q in range(n_ar)
            ]
            xe_full = dram.tile([seg_pad, OUT_CH], mybir.dt.bfloat16)
            xe_red = [
                dram.tile([q_tiles[q] * P, OUT_CH], mybir.dt.bfloat16,
                          name=f"xe_red{q}", addr_space="Shared")
                for q in range(n_ar)
            ]

            # ---- projection: xp_local = bf16(X^T tile' @ wlin) ----
            # xt_shard is feature-major; a [128, 128] column slice is X_tile
            # transposed, exactly the lhsT layout matmul wants.
            QUAD = 4
            for t4 in range(_cdiv(n_tiles_proj, QUAD)):
                nt = min(QUAD, n_tiles_proj - t4 * QUAD)
                xt4 = small.tile([P, QUAD * P], mybir.dt.bfloat16, tag="xt")
                nc.sync.dma_start(
                    xt4[:, :nt * P],
                    xt_shard[:, t4 * QUAD * P:(t4 * QUAD + nt) * P])
                xps4 = small.tile([P, QUAD * P], mybir.dt.bfloat16,
                                  tag="xps")
                for j in range(nt):
                    xpp = psum.tile([P, OUT_CH], mybir.dt.float32,
                                    space="PSUM", tag="xpp")
                    nc.tensor.matmul(xpp[:], xt4[:, j * P:(j + 1) * P],
                                     wlin_sb[:], start=True, stop=True)
                    nc.scalar.activation(xps4[:, j * P:(j + 1) * P], xpp[:],
                                         mybir.ActivationFunctionType.Copy)
                # one quad out-DMA; xp_local rows t4*4P..(t4*4+nt)*P are
                # stored row-major [rows, OUT_CH]; xps4 is [128, nt, 128]
                # with partition = row-within-tile
                nc.sync.dma_start(
                    xp_local[t4 * QUAD * P:(t4 * QUAD + nt) * P, :]
                    .rearrange("(c p) f -> p c f", p=P),
                    xps4[:, :nt * P].rearrange("p (c f) -> p c f", f=P))

            # ---- generic segment-sum over a list of gather calls ----
            def seg_calls(calls, t_lo, t_hi, chunks, co, idx_sb, ids_sb,
                          src_ap, scale_sb, out_fn, out_dtype, ev_tag,
                          hook=None):
                accs = {}
                t_next = t_lo  # next tile with unfinished chunks
                for ci, (co0, nch) in enumerate(calls):
                    if hook is not None:
                        hook(ci)
                    g = work.tile([P, CALL_CHUNKS * P], mybir.dt.bfloat16,
                                  tag="g")
                    gs = g[:, :nch * P].rearrange("p (c e) -> p c e", e=P)
                    nc.gpsimd.dma_gather(
                        gs, src_ap, idx_sb[:, co0 * 8:(co0 + nch) * 8],
                        nch * P, nidx_regs[nch], P,
                        queue_num=qctr[0] % N_QUEUES)
                    qctr[0] += 1
                    s = work.tile([P, CALL_CHUNKS * P], mybir.dt.bfloat16,
                                  tag="s")
                    s3 = s[:, :nch * P].rearrange("p (c e) -> p c e", e=P)
                    nc.vector.tensor_tensor(
                        out=s3,
                        in0=ids_sb[:, co0:co0 + nch].to_broadcast(
                            [P, nch, P]),
                        in1=colidx3.to_broadcast([P, nch, P]),
                        op=mybir.AluOpType.is_equal,
                    )
                    # emit matmuls for every tile overlapping this call
                    t = t_next
                    while t < t_hi and int(co[t]) < co0 + nch:
                        lo = max(int(co[t]), co0)
                        hi = min(int(co[t + 1]), co0 + nch)
                        if t not in accs:
                            accs[t] = psacc.tile([P, OUT_CH],
                                                 mybir.dt.float32,
                                                 space="PSUM", tag="acc",
                                                 name=f"acc_t{t}")
                        acc = accs[t]
                        for j in range(lo, hi):
                            cs = slice((j - co0) * P, (j - co0 + 1) * P)
                            nc.tensor.matmul(
                                acc[:], s[:, cs], g[:, cs],
                                start=(j == int(co[t])),
                                stop=(j == int(co[t + 1]) - 1))
                        if hi == int(co[t + 1]):  # tile finished
                            ev = evp.tile([P, OUT_CH], out_dtype, tag=ev_tag)
                            nc.scalar.activation(
                                ev[:], acc[:],
                                mybir.ActivationFunctionType.Copy,
                                scale=scale_sb[:, t:t + 1])
                            nc.sync.dma_start(out_fn(t), ev[:])
                            del accs[t]
                            t_next = t + 1
                            t += 1
                        else:
                            break
                assert not accs

            # ---- stage 1 (+ chunked AllReduce) ----
            # Each quarter's collective is issued a few calls into the next
            # quarter: by then the quarter's evac DMAs have drained, so the
            # collective's input wait doesn't head-block the gpsimd queue.
            def emit_ar(q):
                if USE_COLLECTIVE:
                    nc.gpsimd.collective_compute(
                        "AllReduce", mybir.AluOpType.add,
                        replica_groups=[list(range(n_cores))],
                        ins=[xe_part[q].opt()],
                        outs=[xe_red[q].opt()],
                    )
                    nc.sync.dma_start(
                        xe_full[q_tile_lo[q] * P:q_tile_lo[q + 1] * P, :],
                        xe_red[q][:])
                else:
                    nc.sync.dma_start(
                        xe_full[q_tile_lo[q] * P:q_tile_lo[q + 1] * P, :],
                        xe_part[q][:])

            for q in range(n_ar):
                def out1(t, q=q):
                    trel = t - int(q_tile_lo[q])
                    return xe_part[q][trel * P:(trel + 1) * P, :]

                ar_ci = min(2, len(calls1[q]) - 1)

                def hook(ci, q=q, ar_ci=ar_ci):
                    if q > 0 and ci == ar_ci:
                        emit_ar(q - 1)

                seg_calls(calls1[q], int(q_tile_lo[q]), int(q_tile_lo[q + 1]),
                          chunks1, co1, idx1_sb, ids1_sb, xp_local[:],
                          scale_e, out1, mybir.dt.bfloat16, "ev1", hook)
                if q == n_ar - 1:
                    emit_ar(q)

            # ---- stage 2 ----
            def out2(t):
                return out_shard[t * P:(t + 1) * P, :]

            seg_calls(calls2, 0, n_node_tiles, chunks2, co2, idx2_sb,
                      ids2_sb, xe_full[:], degv_sb, out2, mybir.dt.float32,
                      "ev2")

    nc.compile()
    return nc


def _host_prep(X, Wlin, degE, degV, W, g1_src, g1_dst, n_cores=N_CORES):
    ns = N_NODES // n_cores
    ns_pad = _cdiv(ns, P) * P
    n_seg_tiles = _cdiv(N_HEDGES, P)
    seg_pad = n_seg_tiles * P
    n_node_tiles = ns_pad // P

    core_of = g1_src // ns

    # stage 1: per core, sorted by dst
    o1 = np.lexsort((g1_dst, core_of))
    src1, dst1, c1 = g1_src[o1], g1_dst[o1], core_of[o1]
    cb1 = np.searchsorted(c1, np.arange(n_cores + 1))
    tile_key1, gidx1, lid1 = [], [], []
    for c in range(n_cores):
        lo, hi = cb1[c], cb1[c + 1]
        d = dst1[lo:hi]
        tile_key1.append(d // P)
        gidx1.append(src1[lo:hi] - c * ns)
        lid1.append((d % P).astype(np.float32))
    chunks1, idx1_w, ids1_w = _prep_stage(
        tile_key1, gidx1, lid1, n_seg_tiles, n_cores)

    # stage 2: per core, sorted by src
    o2 = np.argsort(g1_src, kind="stable")
    src2, dst2 = g1_src[o2], g1_dst[o2]
    cb2 = np.searchsorted(src2, np.arange(n_cores + 1) * ns)
    tile_key2, gidx2, lid2 = [], [], []
    for c in range(n_cores):
        lo, hi = cb2[c], cb2[c + 1]
        s_local = src2[lo:hi] - c * ns
        tile_key2.append(s_local // P)
        gidx2.append(dst2[lo:hi])
        lid2.append((s_local % P).astype(np.float32))
    chunks2, idx2_w, ids2_w = _prep_stage(
        tile_key2, gidx2, lid2, n_node_tiles, n_cores)

    # rearranged scale vectors: column t holds values for tile t's rows;
    # degE is pre-multiplied by W (elementwise hyperedge weight)
    def col_tiles(v, pad_rows):
        vp = np.zeros(pad_rows, dtype=np.float32)
        vp[:v.shape[0]] = v.reshape(-1)
        return np.ascontiguousarray(vp.reshape(pad_rows // P, P).T)

    dege_r = col_tiles((degE * W).astype(np.float32), seg_pad)
    colidx = np.broadcast_to(
        np.arange(P, dtype=np.float32), (P, P)).astype(BF16)

    in_maps = []
    for c in range(n_cores):
        xs = np.zeros((ns_pad, IN_CH), dtype=np.float32)
        xs[:ns] = X[c * ns:(c + 1) * ns]
        in_maps.append({
            "xt_shard": np.ascontiguousarray(xs.T.astype(BF16)),
            "wlin": np.ascontiguousarray(Wlin.astype(BF16)),
            "dege_r": dege_r,
            "degv_r": col_tiles(degV[c * ns:(c + 1) * ns], ns_pad),
            "colidx": np.ascontiguousarray(colidx),
            "idx1": idx1_w[c],
            "ids1": ids1_w[c],
            "idx2": idx2_w[c],
            "ids2": ids2_w[c],
        })
    return in_maps, chunks1, chunks2, ns, ns_pad, seg_pad


def run_impl(inputs: dict, trace: bool = False):
    X = np.asarray(inputs["X"], dtype=np.float32)
    Wlin = np.asarray(inputs["Wlin"], dtype=np.float32)
    degE = np.asarray(inputs["degE"], dtype=np.float32)
    degV = np.asarray(inputs["degV"], dtype=np.float32)
    W = np.asarray(inputs["W"], dtype=np.float32)
    g1_src = np.asarray(inputs["g1_src"], dtype=np.int64)
    g1_dst = np.asarray(inputs["g1_dst"], dtype=np.int64)

    in_maps, chunks1, chunks2, ns, ns_pad, seg_pad = _host_prep(
        X, Wlin, degE, degV, W, g1_src, g1_dst)
    nc = _build_program(ns_pad, seg_pad, chunks1, chunks2, N_CORES)
    res = run_bass_kernel_spmd(nc, in_maps, core_ids=list(range(N_CORES)),
                               trace=trace)
    out = np.concatenate(
        [res.results[c]["out_shard"][:ns] for c in range(N_CORES)], axis=0)
    return out, res


def kernel(**inputs) -> np.ndarray:
    out, _ = run_impl(inputs, trace=False)
    return out


# revision 26
# speedup vs baseline: 1.0165x; 1.0165x over previous
"""Trainium2 Bass kernel for DGL HGNNConv-style hypergraph message passing.

Computation (see problem reference):
    Xp = X @ Wlin                                   # [N, 128] @ [128, 128]
    Xe = segment_sum(Xp[g1_src], g1_dst, 25000)     # node -> hyperedge
    Xe = Xe * degE * W
    Xv = segment_sum(Xe[g1_dst], g1_src, 100000)    # hyperedge -> node
    Xv = Xv * degV

Distribution strategy (8 NeuronCores, node-range sharding):
  - Core m owns node rows [m*12500, (m+1)*12500) and all nnz entries whose
    src falls in that range (both stages use the same entry sharding).
  - Projection: each core computes Xp (bf16) for its own node shard only.
    The host ships X pre-transposed (feature-major) in bf16, so projection
    is a plain matmul per tile (no on-chip transpose or cast).
  - Stage 1: per-core entries sorted by dst; rows of the local Xp gathered
    per entry (dma_gather), segment-summed into a full-range partial Xe
    via data-dependent one-hot matmuls (PSUM accumulation), scaled by
    degE*W, then AllReduced across cores (4 chunks, overlapped).
  - Stage 2: per-core entries sorted by src; rows of the reduced Xe
    gathered per entry, segment-summed into the core's node tile,
    scaled by degV, written to the core's output shard.

Performance notes:
  - SWDGE descriptor generation on the Pool engine costs ~1us fixed per
    dma_gather call; calls are packed to exactly CALL_CHUNKS chunks
    (spanning output-tile boundaries), the per-call maximum imposed by the
    ucode's 128-entry in-flight descriptor-ring limit (each ring entry
    covers 16 rows, both directions counted: 7*128/16*2 = 114 <= 128).
    Larger calls deadlock the device.
  - Gather tables (Xp, reduced Xe) are stored bf16: halves gather HBM
    traffic and feeds the one-hot matmuls bf16 inputs directly.
  - One-hot build (is_equal) runs on the vector engine, one instruction
    per gather call; PSUM evacuation + deg scaling runs fused on the
    (otherwise idle) scalar engine via activation(Copy, scale=...).
  - PSUM accumulator chains span gather calls: a tile's matmul chain
    continues into the next call's chunks (start/stop flags bracket the
    whole tile).
  - The AllReduce runs in 3 chunks ([64, 88, 44] seg tiles), each emitted
    2 gather calls into the NEXT chunk's quarter so its link time hides
    behind that quarter's remaining gather stream (shape and emission
    point must be tuned together). Each chunk lands in its own Shared
    tensor (the framework allows only one writer per Shared tensor) and
    is copied into the contiguous xe_full gather table off the critical
    path.

Segment-sum-as-matmul: for each chunk of 128 gathered rows G [128e x 128f]
and one-hot S [128e x 128s] (S[k, m] = 1 iff entry k belongs to local
segment m, built on-chip with is_equal against an iota tile), the matmul
S^T @ G accumulates the chunk into the 128-segment PSUM tile.
"""

import ml_dtypes
import numpy as np

import concourse.bass as bass
import concourse.bacc as bacc
import concourse.tile as tile
import concourse.mybir as mybir
from concourse.bass_utils import run_bass_kernel_spmd

P = 128
N_CORES = 8
N_QUEUES = 4

N_NODES = 100000
N_HEDGES = 25000
IN_CH = 128
OUT_CH = 128
N_AR_CHUNKS = 3  # AllReduce split for overlap with stage-1 compute
USE_COLLECTIVE = True
CALL_CHUNKS = 7  # chunks (x128 rows) per dma_gather call (in-flight limit)

BF16 = ml_dtypes.bfloat16


def _cdiv(a, b):
    return (a + b - 1) // b


def _wrap_idx16(idx_flat: np.ndarray) -> np.ndarray:
    """Pack a flat index array into the [128, n/16] int16 SBUF layout used
    by dma_gather: flat index i -> partition i%16, column i//16, replicated
    across the eight 16-partition stripes."""
    n = idx_flat.shape[0]
    assert n % 16 == 0
    blk = idx_flat.astype(np.int16).reshape(n // 16, 16).T  # [16, cols]
    return np.tile(blk, (8, 1))  # [128, cols]


def _prep_stage(tile_key, gather_idx, local_id, n_tiles, n_cores):
    """Build per-core padded gather-index / segment-id arrays with a chunk
    schedule that is uniform across cores (SPMD requires one program).

    tile_key: per-core arrays with the tile id per entry (nondecreasing).
    Returns (chunks [n_tiles], idx_wrapped list, ids list).
    """
    counts = np.zeros((n_cores, n_tiles), dtype=np.int64)
    slices = []
    for c in range(n_cores):
        bounds = np.searchsorted(tile_key[c], np.arange(n_tiles + 1),
                                 side="left")
        counts[c] = bounds[1:] - bounds[:-1]
        slices.append(bounds)
    chunks = np.maximum(1, _cdiv(counts.max(axis=0), P)).astype(np.int64)
    total_chunks = int(chunks.sum())
    total = total_chunks * P
    co = np.concatenate([[0], np.cumsum(chunks)])

    idx_w, ids_w = [], []
    for c in range(n_cores):
        idx_flat = np.zeros(total, dtype=np.int16)
        ids_flat = np.full(total, -1.0, dtype=np.float32)
        bounds = slices[c]
        gi, li = gather_idx[c], local_id[c]
        for t in range(n_tiles):
            lo, hi = bounds[t], bounds[t + 1]
            cnt = hi - lo
            base = int(co[t]) * P
            idx_flat[base:base + cnt] = gi[lo:hi]
            ids_flat[base:base + cnt] = li[lo:hi]
        idx_w.append(_wrap_idx16(idx_flat))
        ids_w.append(np.ascontiguousarray(
            ids_flat.reshape(total_chunks, P).T).astype(BF16))
    return chunks, idx_w, ids_w


def _make_calls(chunk_lo, chunk_hi, max_chunks):
    """Pack the chunk range [chunk_lo, chunk_hi) into gather calls of at
    most max_chunks chunks (full-size except the last)."""
    calls = []
    c = chunk_lo
    while c < chunk_hi:
        n = min(max_chunks, chunk_hi - c)
        calls.append((c, n))
        c += n
    return calls


def _build_program(ns_pad, seg_pad, chunks1, chunks2, n_cores):
    """Emit the SPMD Bass program (identical for all cores)."""
    n_tiles_proj = ns_pad // P
    n_seg_tiles = seg_pad // P
    n_node_tiles = ns_pad // P
    tc1 = int(chunks1.sum())
    tc2 = int(chunks2.sum())
    co1 = np.concatenate([[0], np.cumsum(chunks1)]).astype(int)
    co2 = np.concatenate([[0], np.cumsum(chunks2)]).astype(int)

    nc = bacc.Bacc("TRN2", target_bir_lowering=False, debug=False,
                   num_devices=n_cores, num_swdge_queues=N_QUEUES)

    xt_shard = nc.dram_tensor("xt_shard", [IN_CH, ns_pad], mybir.dt.bfloat16,
                              kind="ExternalInput")
    wlin = nc.dram_tensor("wlin", [IN_CH, OUT_CH], mybir.dt.bfloat16,
                          kind="ExternalInput")
    dege_r = nc.dram_tensor("dege_r", [P, n_seg_tiles], mybir.dt.float32,
                            kind="ExternalInput")
    degv_r = nc.dram_tensor("degv_r", [P, n_node_tiles], mybir.dt.float32,
                            kind="ExternalInput")
    colidx_in = nc.dram_tensor("colidx", [P, P], mybir.dt.bfloat16,
                               kind="ExternalInput")
    idx1_in = nc.dram_tensor("idx1", [P, tc1 * 8], mybir.dt.int16,
                             kind="ExternalInput")
    ids1_in = nc.dram_tensor("ids1", [P, tc1], mybir.dt.bfloat16,
                             kind="ExternalInput")
    idx2_in = nc.dram_tensor("idx2", [P, tc2 * 8], mybir.dt.int16,
                             kind="ExternalInput")
    ids2_in = nc.dram_tensor("ids2", [P, tc2], mybir.dt.bfloat16,
                             kind="ExternalInput")
    out_shard = nc.dram_tensor("out_shard", [ns_pad, OUT_CH],
                               mybir.dt.float32, kind="ExternalOutput")

    # AllReduce chunk row ranges (in seg tiles): front-loaded groups with a
    # small final group so the pipeline-drain tail before the last AllReduce
    # is short
    n_ar = min(N_AR_CHUNKS, n_seg_tiles)
    if n_ar == 3 and n_seg_tiles == 196:
        q_tiles = [64, 88, 44]
    elif n_ar == 4 and n_seg_tiles == 196:
        q_tiles = [60, 60, 60, 16]
    else:
        q_tiles = [n_seg_tiles // n_ar] * n_ar
        for i in range(n_seg_tiles % n_ar):
            q_tiles[i] += 1
    q_tile_lo = np.concatenate([[0], np.cumsum(q_tiles)]).astype(int)

    # gather calls: exactly CALL_CHUNKS chunks each (spanning tile
    # boundaries), aligned to AR quarters in stage 1
    calls1 = [
        _make_calls(int(co1[q_tile_lo[q]]), int(co1[q_tile_lo[q + 1]]),
                    CALL_CHUNKS)
        for q in range(n_ar)
    ]
    calls2 = _make_calls(0, tc2, CALL_CHUNKS)
    call_sizes = sorted(
        {n for qc in calls1 for (_, n) in qc} | {n for (_, n) in calls2})

    qctr = [0]  # SWDGE queue rotation

    with tile.TileContext(nc) as tc:
        with (
            tc.tile_pool(name="const", bufs=1) as cpool,
            tc.tile_pool(name="work", bufs=16) as work,
            tc.tile_pool(name="evp", bufs=12) as evp,
            tc.tile_pool(name="small", bufs=6) as small,
            tc.tile_pool(name="psum", bufs=2, space="PSUM") as psum,
            tc.tile_pool(name="psacc", bufs=6, space="PSUM") as psacc,
            tc.tile_pool(name="dram", bufs=1, space="DRAM") as dram,
        ):
            # ---- preloads ----
            idx1_sb = cpool.tile([P, tc1 * 8], mybir.dt.int16)
            nc.sync.dma_start(idx1_sb[:], idx1_in[:])
            ids1_sb = cpool.tile([P, tc1], mybir.dt.bfloat16)
            nc.sync.dma_start(ids1_sb[:], ids1_in[:])
            idx2_sb = cpool.tile([P, tc2 * 8], mybir.dt.int16)
            nc.sync.dma_start(idx2_sb[:], idx2_in[:])
            ids2_sb = cpool.tile([P, tc2], mybir.dt.bfloat16)
            nc.sync.dma_start(ids2_sb[:], ids2_in[:])
            colidx_sb = cpool.tile([P, P], mybir.dt.bfloat16)
            nc.sync.dma_start(colidx_sb[:], colidx_in[:])
            wlin_sb = cpool.tile([P, OUT_CH], mybir.dt.bfloat16)
            nc.sync.dma_start(wlin_sb[:], wlin[:])
            degv_sb = cpool.tile([P, n_node_tiles], mybir.dt.float32)
            nc.sync.dma_start(degv_sb[:], degv_r[:])
            scale_e = cpool.tile([P, n_seg_tiles], mybir.dt.float32)
            nc.sync.dma_start(scale_e[:], dege_r[:])
            colidx3 = colidx_sb[:].rearrange("p (o e) -> p o e", o=1)
            # pre-staged num_idxs registers (a MOVE occupies a scarce gpsimd
            # exec-queue slot, so these are hoisted out of the loops)
            nidx_regs = {n: nc.gpsimd.to_reg(n * P) for n in call_sizes}

            xp_local = dram.tile([ns_pad, OUT_CH], mybir.dt.bfloat16)
            xe_part = [
                dram.tile([q_tiles[q] * P, OUT_CH], mybir.dt.bfloat16,
                          name=f"xe_part{q}")
                for q in range(n_ar)
            ]
            xe_full = dram.tile([seg_pad, OUT_CH], mybir.dt.bfloat16)
            xe_red = [
                dram.tile([q_tiles[q] * P, OUT_CH], mybir.dt.bfloat16,
                          name=f"xe_red{q}", addr_space="Shared")
                for q in range(n_ar)
            ]

            # ---- projection: xp_local = bf16(X^T tile' @ wlin) ----
            # xt_shard is feature-major; a [128, 128] column slice is X_tile
            # transposed, exactly the lhsT layout matmul wants.
            QUAD = 4
            for t4 in range(_cdiv(n_tiles_proj, QUAD)):
                nt = min(QUAD, n_tiles_proj - t4 * QUAD)
                xt4 = small.tile([P, QUAD * P], mybir.dt.bfloat16, tag="xt")
                nc.sync.dma_start(
                    xt4[:, :nt * P],
                    xt_shard[:, t4 * QUAD * P:(t4 * QUAD + nt) * P])
                xps4 = small.tile([P, QUAD * P], mybir.dt.bfloat16,
                                  tag="xps")
                for j in range(nt):
                    xpp = psum.tile([P, OUT_CH], mybir.dt.float32,
                                    space="PSUM", tag="xpp")
                    nc.tensor.matmul(xpp[:], xt4[:, j * P:(j + 1) * P],
                                     wlin_sb[:], start=True, stop=True)
                    nc.scalar.activation(xps4[:, j * P:(j + 1) * P], xpp[:],
                                         mybir.ActivationFunctionType.Copy)
                # one quad out-DMA; xp_local rows t4*4P..(t4*4+nt)*P are
                # stored row-major [rows, OUT_CH]; xps4 is [128, nt, 128]
                # with partition = row-within-tile
                nc.sync.dma_start(
                    xp_local[t4 * QUAD * P:(t4 * QUAD + nt) * P, :]
                    .rearrange("(c p) f -> p c f", p=P),
                    xps4[:, :nt * P].rearrange("p (c f) -> p c f", f=P))

            # ---- generic segment-sum over a list of gather calls ----
            def seg_calls(calls, t_lo, t_hi, chunks, co, idx_sb, ids_sb,
                          src_ap, scale_sb, out_fn, out_dtype, ev_tag,
                          hook=None):
                accs = {}
                t_next = t_lo  # next tile with unfinished chunks
                for ci, (co0, nch) in enumerate(calls):
                    if hook is not None:
                        hook(ci)
                    g = work.tile([P, CALL_CHUNKS * P], mybir.dt.bfloat16,
                                  tag="g")
                    gs = g[:, :nch * P].rearrange("p (c e) -> p c e", e=P)
                    nc.gpsimd.dma_gather(
                        gs, src_ap, idx_sb[:, co0 * 8:(co0 + nch) * 8],
                        nch * P, nidx_regs[nch], P,
                        queue_num=qctr[0] % N_QUEUES)
                    qctr[0] += 1
                    s = work.tile([P, CALL_CHUNKS * P], mybir.dt.bfloat16,
                                  tag="s")
                    s3 = s[:, :nch * P].rearrange("p (c e) -> p c e", e=P)
                    nc.vector.tensor_tensor(
                        out=s3,
                        in0=ids_sb[:, co0:co0 + nch].to_broadcast(
                            [P, nch, P]),
                        in1=colidx3.to_broadcast([P, nch, P]),
                        op=mybir.AluOpType.is_equal,
                    )
                    # emit matmuls for every tile overlapping this call
                    t = t_next
                    while t < t_hi and int(co[t]) < co0 + nch:
                        lo = max(int(co[t]), co0)
                        hi = min(int(co[t + 1]), co0 + nch)
                        if t not in accs:
                            accs[t] = psacc.tile([P, OUT_CH],
                                                 mybir.dt.float32,
                                                 space="PSUM", tag="acc",
                                                 name=f"acc_t{t}")
                        acc = accs[t]
                        for j in range(lo, hi):
                            cs = slice((j - co0) * P, (j - co0 + 1) * P)
                            nc.tensor.matmul(
                                acc[:], s[:, cs], g[:, cs],
                                start=(j == int(co[t])),
                                stop=(j == int(co[t + 1]) - 1))
                        if hi == int(co[t + 1]):  # tile finished
                            ev = evp.tile([P, OUT_CH], out_dtype, tag=ev_tag)
                            nc.scalar.activation(
                                ev[:], acc[:],
                                mybir.ActivationFunctionType.Copy,
                                scale=scale_sb[:, t:t + 1])
                            nc.sync.dma_start(out_fn(t), ev[:])
                            del accs[t]
                            t_next = t + 1
                            t += 1
                        else:
                            break
                assert not accs

            # ---- stage 1 (+ chunked AllReduce) ----
            # Each quarter's collective is issued a few calls into the next
            # quarter: by then the quarter's evac DMAs have drained, so the
            # collective's input wait doesn't head-block the gpsimd queue.
            def emit_ar(q):
                if USE_COLLECTIVE:
                    nc.gpsimd.collective_compute(
                        "AllReduce", mybir.AluOpType.add,
                        replica_groups=[list(range(n_cores))],
                        ins=[xe_part[q].opt()],
                        outs=[xe_red[q].opt()],
                    )
                    nc.sync.dma_start(
                        xe_full[q_tile_lo[q] * P:q_tile_lo[q + 1] * P, :],
                        xe_red[q][:])
                else:
                    nc.sync.dma_start(
                        xe_full[q_tile_lo[q] * P:q_tile_lo[q + 1] * P, :],
                        xe_part[q][:])

            for q in range(n_ar):
                def out1(t, q=q):
                    trel = t - int(q_tile_lo[q])
                    return xe_part[q][trel * P:(trel + 1) * P, :]

                ar_ci = max(2, len(calls1[q]) // 2)

                def hook(ci, q=q, ar_ci=ar_ci):
                    if q > 0 and ci == ar_ci:
                        emit_ar(q - 1)

                seg_calls(calls1[q], int(q_tile_lo[q]), int(q_tile_lo[q + 1]),
                          chunks1, co1, idx1_sb, ids1_sb, xp_local[:],
                          scale_e, out1, mybir.dt.bfloat16, "ev1", hook)
                if q == n_ar - 1:
                    emit_ar(q)

            # ---- stage 2 ----
            def out2(t):
                return out_shard[t * P:(t + 1) * P, :]

            seg_calls(calls2, 0, n_node_tiles, chunks2, co2, idx2_sb,
                      ids2_sb, xe_full[:], degv_sb, out2, mybir.dt.float32,
                      "ev2")

    nc.compile()
    return nc


def _host_prep(X, Wlin, degE, degV, W, g1_src, g1_dst, n_cores=N_CORES):
    ns = N_NODES // n_cores
    ns_pad = _cdiv(ns, P) * P
    n_seg_tiles = _cdiv(N_HEDGES, P)
    seg_pad = n_seg_tiles * P
    n_node_tiles = ns_pad // P

    core_of = g1_src // ns

    # stage 1: per core, sorted by dst
    o1 = np.lexsort((g1_dst, core_of))
    src1, dst1, c1 = g1_src[o1], g1_dst[o1], core_of[o1]
    cb1 = np.searchsorted(c1, np.arange(n_cores + 1))
    tile_key1, gidx1, lid1 = [], [], []
    for c in range(n_cores):
        lo, hi = cb1[c], cb1[c + 1]
        d = dst1[lo:hi]
        tile_key1.append(d // P)
        gidx1.append(src1[lo:hi] - c * ns)
        lid1.append((d % P).astype(np.float32))
    chunks1, idx1_w, ids1_w = _prep_stage(
        tile_key1, gidx1, lid1, n_seg_tiles, n_cores)

    # stage 2: per core, sorted by src
    o2 = np.argsort(g1_src, kind="stable")
    src2, dst2 = g1_src[o2], g1_dst[o2]
    cb2 = np.searchsorted(src2, np.arange(n_cores + 1) * ns)
    tile_key2, gidx2, lid2 = [], [], []
    for c in range(n_cores):
        lo, hi = cb2[c], cb2[c + 1]
        s_local = src2[lo:hi] - c * ns
        tile_key2.append(s_local // P)
        gidx2.append(dst2[lo:hi])
        lid2.append((s_local % P).astype(np.float32))
    chunks2, idx2_w, ids2_w = _prep_stage(
        tile_key2, gidx2, lid2, n_node_tiles, n_cores)

    # rearranged scale vectors: column t holds values for tile t's rows;
    # degE is pre-multiplied by W (elementwise hyperedge weight)
    def col_tiles(v, pad_rows):
        vp = np.zeros(pad_rows, dtype=np.float32)
        vp[:v.shape[0]] = v.reshape(-1)
        return np.ascontiguousarray(vp.reshape(pad_rows // P, P).T)

    dege_r = col_tiles((degE * W).astype(np.float32), seg_pad)
    colidx = np.broadcast_to(
        np.arange(P, dtype=np.float32), (P, P)).astype(BF16)

    in_maps = []
    for c in range(n_cores):
        xs = np.zeros((ns_pad, IN_CH), dtype=np.float32)
        xs[:ns] = X[c * ns:(c + 1) * ns]
        in_maps.append({
            "xt_shard": np.ascontiguousarray(xs.T.astype(BF16)),
            "wlin": np.ascontiguousarray(Wlin.astype(BF16)),
            "dege_r": dege_r,
            "degv_r": col_tiles(degV[c * ns:(c + 1) * ns], ns_pad),
            "colidx": np.ascontiguousarray(colidx),
            "idx1": idx1_w[c],
            "ids1": ids1_w[c],
            "idx2": idx2_w[c],
            "ids2": ids2_w[c],
        })
    return in_maps, chunks1, chunks2, ns, ns_pad, seg_pad


def run_impl(inputs: dict, trace: bool = False):
    X = np.asarray(inputs["X"], dtype=np.float32)
    Wlin = np.asarray(inputs["Wlin"], dtype=np.float32)
    degE = np.asarray(inputs["degE"], dtype=np.float32)
    degV = np.asarray(inputs["degV"], dtype=np.float32)
    W = np.asarray(inputs["W"], dtype=np.float32)
    g1_src = np.asarray(inputs["g1_src"], dtype=np.int64)
    g1_dst = np.asarray(inputs["g1_dst"], dtype=np.int64)

    in_maps, chunks1, chunks2, ns, ns_pad, seg_pad = _host_prep(
        X, Wlin, degE, degV, W, g1_src, g1_dst)
    nc = _build_program(ns_pad, seg_pad, chunks1, chunks2, N_CORES)
    res = run_bass_kernel_spmd(nc, in_maps, core_ids=list(range(N_CORES)),
                               trace=trace)
    out = np.concatenate(
        [res.results[c]["out_shard"][:ns] for c in range(N_CORES)], axis=0)
    return out, res


def kernel(**inputs) -> np.ndarray:
    out, _ = run_impl(inputs, trace=False)
    return out
